# revision 24
# baseline (speedup 1.0000x reference)
"""Trainium2 Bass kernel for nn_CompositeEmbeddingA (octree composite embedding).

Batch=8, one sample per NeuronCore (pure data parallel, no collectives).

Per sample:
  layers 0-2 (depths 1-3): x = val_emb[v] + pos0[p0] + pos1[p1] + pos2[p2] + dep_emb[d]
  layers 3-4: same sum w/o dep, then Conv1d(E,E,kernel=stride=k), k=4 (l3) / 8 (l4)

Formulation: every layer is out = OneHot @ Table on the PE.
  - depth is static per layer -> dep_emb row folded into val_emb rows.
  - conv folded into tables per tap j (T_j = table @ w[:,:,j].T); conv bias
    folded into tap-0 val rows (exactly one val row fires per tap).
  - index rows 0 of all tables are never referenced (indices are >= 1) and
    are dropped: 192 rows per (sub-layer | tap), so
    merged layers 0-2 ("B"): 576 rows -> 5 chunks of 128
    layer 3: 4 taps  -> 768 rows  -> 6 chunks
    layer 4: 8 taps  -> 1536 rows -> 12 chunks
  - the one-hot matrix (pure index preprocessing, no table data) is built
    host-side and shipped as fp8 (exact 0/1) in the DoubleRow k-tile layout.
  - tables are shipped as residual-compensated fp8 pairs (A = fp8(T/S),
    B = fp8(T/S - A)); each chunk is one fp8 DoubleRow matmul contracting
    (A, B) against the same one-hot (stride-0 k-tile broadcast), which costs
    half of a bf16 matmul per chunk at bf16-level accuracy (~1e-3 rel).
  - PSUM is evicted to bf16 with scale S, rotating DVE/ACT/Pool; output DMAs
    in bf16, host casts to f32 and reassembles token tiles.
"""

import sys

for _p in ("/opt/trn_rl_repo",):
    if _p not in sys.path:
        sys.path.insert(0, _p)

import numpy as np
import ml_dtypes

RES = 32
SPATIAL = 3
NUM_VOCAB = 3
E = 256
BATCH = 8
LAYER_SIZES = (8, 64, 512, 4096, 32768)
CONV_SIZE = {3: 4, 4: 8}
S_TOTAL = sum(LAYER_SIZES)  # 37448
OUT_TOKENS = 8 + 64 + 512 + 1024 + 4096  # 5704

_E8 = ml_dtypes.float8_e4m3fn
_BF16 = ml_dtypes.bfloat16

SCALE = 2.0**-9  # global table scale; folded back in at PSUM evict

ROWS_PER_GROUP = 3 + 63 * SPATIAL  # 192: val(3) + pos0/1/2(63 each)

# virtual layers: (name, token count T, n 128-row chunks, list of t-tile sizes)
def _tiles(T):
    return [min(128, T - t0) for t0 in range(0, T, 128)]

VLAYERS = (
    ("B", 584, 5),     # sub-layers 0-2 merged; 576 rows
    ("L3", 1024, 6),   # 768 rows
    ("L4", 4096, 12),  # 1536 rows
)
NCH_TOTAL = sum(nch for _, _, nch in VLAYERS)  # 23
NTT_TOTAL = sum(len(_tiles(T)) for _, T, _ in VLAYERS)  # 45
L4_STRIPE = 512  # tokens per L4 MH load


def _layer_slices():
    out = []
    start = 0
    for n in LAYER_SIZES:
        out.append((start, start + n))
        start += n
    return out


LAYER_SL = _layer_slices()


def _build_tables(params):
    """Residual-compensated fp8 table chunks.

    Returns tbl [128, NCH_TOTAL, 2, E] (fp8): per chunk, k-tile 0 = coarse
    fp8(T/S), k-tile 1 = fp8 residual.
    """
    rows = []

    def add_group(val3, pe):  # val3 [3,E], pe [SPATIAL, 64, E]
        rows.append(val3)
        for s in range(SPATIAL):
            rows.append(pe[s][1:64])

    # B: sub-layers 0..2, dep folded into val
    for l in range(3):
        val3 = (
            np.asarray(params[f"val_emb_{l}"], np.float32)[1:4]
            + np.asarray(params[f"dep_emb_{l}"], np.float32)[l + 1][None, :]
        )
        add_group(val3, np.asarray(params[f"pos_emb_{l}"], np.float32))
    # conv layers: per tap, tables folded through w[:,:,j]; bias into tap-0 val
    for l in (3, 4):
        w = np.asarray(params[f"conv_w_{l}"], np.float32)  # [O, E, k]
        b = np.asarray(params[f"conv_b_{l}"], np.float32)
        ve = np.asarray(params[f"val_emb_{l}"], np.float32)
        pe = np.asarray(params[f"pos_emb_{l}"], np.float32)
        for j in range(CONV_SIZE[l]):
            wj = w[:, :, j]
            val3 = ve[1:4] @ wj.T
            if j == 0:
                val3 = val3 + b[None, :]
            add_group(val3, pe @ wj.T)

    allrows = np.concatenate(rows, axis=0)  # [2880, E]
    assert allrows.shape[0] == 576 + 768 + 1536

    tbl = np.zeros((128, NCH_TOTAL, 2, E), np.float32)
    r0 = 0
    c0 = 0
    for _, _, nch in VLAYERS:
        n = {5: 576, 6: 768, 12: 1536}[nch]
        lay = np.zeros((nch * 128, E), np.float32)
        lay[:n] = allrows[r0 : r0 + n]
        r0 += n
        t = lay.reshape(nch, 128, E).transpose(1, 0, 2)  # [128, nch, E]
        a = (t / SCALE).astype(_E8).astype(np.float32)
        resid = (t / SCALE - a).astype(_E8).astype(np.float32)
        tbl[:, c0 : c0 + nch, 0, :] = a
        tbl[:, c0 : c0 + nch, 1, :] = resid
        c0 += nch
    return tbl.astype(_E8)


def _build_mh(value, position, b):
    """Host-built one-hot planes, fp8, chunk-major columns per virtual layer.

    Returns dict name -> [128, nch * T] fp8 where column c*T + t is chunk c,
    token t; row r fires iff global row id c*128+r is selected by token t.
    """
    out = {}

    def onehot(pairs, T, nch):
        # pairs: list of (gid_array, col_array); sets m[gid, col] = 1
        m = np.zeros((nch * 128, T), _E8)
        one = _E8(1.0)
        for g, c in pairs:
            m[g, c] = one
        return np.ascontiguousarray(
            m.reshape(nch, 128, T).transpose(1, 0, 2).reshape(128, nch * T)
        )

    # B: merged sub-layers; token cols 0..583 == input tokens 0..583
    pairs = []
    for l in range(3):
        lo, hi = LAYER_SL[l]
        cols = np.arange(lo, hi)
        base = ROWS_PER_GROUP * l
        pairs.append((base + (value[b, lo:hi] - 1), cols))
        for s in range(SPATIAL):
            pairs.append((base + 3 + 63 * s + (position[b, lo:hi, s] - 1), cols))
    out["B"] = onehot(pairs, 584, 5)

    for name, l in (("L3", 3), ("L4", 4)):
        k = CONV_SIZE[l]
        lo, hi = LAYER_SL[l]
        T = (hi - lo) // k
        nch = 6 if l == 3 else 12
        cols = np.arange(T)
        pairs = []
        for j in range(k):
            base = ROWS_PER_GROUP * j
            pairs.append((base + (value[b, lo:hi][j::k] - 1), cols))
            for s in range(SPATIAL):
                pairs.append((base + 3 + 63 * s + (position[b, lo:hi, s][j::k] - 1), cols))
        out[name] = onehot(pairs, T, nch)
    return out


_CACHE = {}


def _get_nc():
    key = "v2"
    if key in _CACHE:
        return _CACHE[key]

    import concourse.bass as bass
    import concourse.tile as tile
    from concourse import bacc, mybir
    from contextlib import ExitStack

    f32 = mybir.dt.float32
    bf16 = mybir.dt.bfloat16
    fp8 = mybir.dt.float8e4
    DR = mybir.MatmulPerfMode.DoubleRow
    A = mybir.ActivationFunctionType

    nc = bacc.Bacc(trn_type="TRN2", target_bir_lowering=False, debug=False)

    tbl_d = nc.dram_tensor("tbl", [128, NCH_TOTAL * 2 * E], fp8, kind="ExternalInput").ap()
    mh_d = {
        name: nc.dram_tensor(f"mh_{name}", [128, nch * T], fp8, kind="ExternalInput").ap()
        for name, T, nch in VLAYERS
    }
    out_d = nc.dram_tensor("out", [128, NTT_TOTAL * E], bf16, kind="ExternalOutput").ap()

    # chunk offset of each vlayer in tbl
    coff = {}
    c0 = 0
    for name, _, nch in VLAYERS:
        coff[name] = c0
        c0 += nch

    with tile.TileContext(nc) as tc, ExitStack() as ctx:
        cpool = ctx.enter_context(tc.tile_pool(name="const", bufs=1))
        psum = ctx.enter_context(tc.tile_pool(name="ps", bufs=4, space=bass.MemorySpace.PSUM))
        opool = ctx.enter_context(tc.tile_pool(name="osb", bufs=1))

        tbl_t = cpool.tile([128, NCH_TOTAL, 2, E], fp8, tag="tbl", name="tbl_t")
        tbl_v = tbl_d[:].rearrange("p (c k e) -> p c k e", k=2, e=E)

        # rotate DMAs over the available queues so HWDGE/SWDGE descriptor
        # generation overlaps other queues' transfers
        def dma_in(dst, src):
            nc.sync.dma_start(dst, src)

        def dma_out(dst, src):
            nc.scalar.dma_start(dst, src)

        def load_tbl(lo, hi):
            dma_in(tbl_t[:, lo:hi], tbl_v[:, lo:hi])

        # MH stripes: (vlayer name, token start, width, sbuf tile)
        mh_tiles = {}

        def load_mh(name, s0, W):
            T = dict((n, t) for n, t, _ in VLAYERS)[name]
            nch = dict((n, c) for n, _, c in VLAYERS)[name]
            t_ = cpool.tile([128, nch, W], fp8, tag=f"mh_{name}_{s0}", name="mh_t")
            src = mh_d[name][:].rearrange("p (c t) -> p c t", t=T)[:, :, s0 : s0 + W]
            dma_in(t_[:], src)
            mh_tiles[(name, s0)] = t_

        # ---- DMA emission order (SP queue FIFO == transfer order) ----
        # small pieces first so PE starts early; L4 stripes stream behind
        load_tbl(0, coff["L3"])  # B tables
        load_mh("B", 0, 584)
        load_tbl(coff["L3"], coff["L4"])  # L3 tables
        load_mh("L3", 0, 1024)
        load_tbl(coff["L4"], coff["L4"] + 12)
        L4_STRIPES = [(0, 512), (512, 512), (1024, 512), (1536, 512),
                      (2048, 512), (2560, 512), (3072, 512), (3584, 384), (3968, 128)]
        for s0, W in L4_STRIPES:
            load_mh("L4", s0, W)

        # ---- compute ----
        # global t-tile index -> out column space
        evict_rr = [0]
        ENGS = ("vector", "scalar")  # gpsimd cannot access PSUM (BIR verifier)

        def evict(dst_ap, src_ap):
            eng = ENGS[evict_rr[0] % len(ENGS)]
            evict_rr[0] += 1
            if eng == "scalar":
                nc.scalar.activation(dst_ap, src_ap, A.Copy, scale=float(SCALE))
            elif eng == "vector":
                nc.vector.tensor_scalar(dst_ap, src_ap, float(SCALE), None, op0=mybir.AluOpType.mult)
            else:
                nc.gpsimd.tensor_scalar(dst_ap, src_ap, float(SCALE), None, op0=mybir.AluOpType.mult)

        gtile = [0]  # global t-tile counter (out column block index)

        # single resident out staging buffer; evicts write slices, a few big
        # DMAs flush column ranges
        ob = opool.tile([128, NTT_TOTAL * E], bf16, tag="ob", name="ob")
        # B's ragged last tile (72 rows) leaves staging rows 72:128 of its
        # block unwritten; zero them once so merged flushes read defined data
        nc.gpsimd.memset(ob[:, 4 * E : 5 * E], 0.0)

        # PE p-state warmup: dummy matmuls keep the tensor engine busy from
        # t~0.5us so it is at full clock when the first real inputs land
        warm = cpool.tile([128, 2, 256], fp8, tag="warm", name="warm")
        nc.gpsimd.memset(warm[:], 0.0)
        wps = psum.tile([128, 256], f32, tag="warmp", name="wps")
        for _ in range(72):
            nc.tensor.matmul(wps[:], warm[:, :, :128], warm[:], start=True, stop=True, perf_mode=DR)

        def do_layer(name, s0, W):
            """mains+evicts for one loaded MH stripe."""
            nch = dict((n, c) for n, _, c in VLAYERS)[name]
            mh = mh_tiles[(name, s0)]
            tiles = _tiles(W)
            ti = 0
            while ti < len(tiles):
                pair = tiles[ti : ti + 2]
                pt = psum.tile([128, 512], f32, tag="o", name="pt")
                for h, M in enumerate(pair):
                    t0 = ti * 128 + h * 128
                    for c in range(nch):
                        lhs = mh[:, c : c + 1, t0 : t0 + M].broadcast_to((128, 2, M))
                        nc.tensor.matmul(
                            pt[:M, h * E : (h + 1) * E],
                            lhs,
                            tbl_t[:, coff[name] + c],
                            start=(c == 0),
                            stop=(c == nch - 1),
                            perf_mode=DR,
                        )
                col = gtile[0] * E
                if len(pair) == 2 and pair[0] == 128 and pair[1] == 128:
                    evict(ob[:, col : col + 2 * E], pt[:])
                else:
                    for h, M in enumerate(pair):
                        evict(ob[:M, col + h * E : col + (h + 1) * E], pt[:M, h * E : (h + 1) * E])
                gtile[0] += len(pair)
                ti += len(pair)

        def flush(g0, g1):
            dma_out(out_d[:, g0 * E : g1 * E], ob[:, g0 * E : g1 * E])

        # compute order matches DMA arrival: B, L3, then L4 stripes
        do_layer("B", 0, 584)
        do_layer("L3", 0, 1024)
        flush(0, 13)  # B (5 tiles incl ragged, hole zeroed) + L3 (8 tiles)
        for i, (s0, W) in enumerate(L4_STRIPES):
            g_before = gtile[0]
            do_layer("L4", s0, W)
            if i in (1, 3, 5):
                flush(gtile[0] - 8, gtile[0])
            elif i >= 6:
                flush(g_before, gtile[0])

    nc.compile()
    _CACHE[key] = nc
    return nc


def kernel(**inputs):
    from concourse.bass_utils import run_bass_kernel_spmd

    value = np.asarray(inputs["value"], np.int64)
    position = np.asarray(inputs["position"], np.int64)

    tbl = _build_tables(inputs)
    tbl_flat = np.ascontiguousarray(tbl.reshape(128, NCH_TOTAL * 2 * E))
    nc = _get_nc()

    in_maps = []
    for b in range(BATCH):
        mh = _build_mh(value, position, b)
        m = {"tbl": tbl_flat}
        for name, T, nch in VLAYERS:
            m[f"mh_{name}"] = mh[name]
        in_maps.append(m)

    res = run_bass_kernel_spmd(nc, in_maps, list(range(BATCH)))

    outs = []
    for b in range(BATCH):
        o = np.asarray(res.results[b]["out"]).astype(np.float32)  # [128, 45*E]
        o = o.reshape(128, NTT_TOTAL, E).transpose(1, 0, 2)  # [45, 128, E]
        # global tile order: B(5) L3(8) L4(32)
        b_tiles = o[0:5].reshape(-1, E)[:584]
        l3_tiles = o[5:13].reshape(-1, E)[:1024]
        l4_tiles = o[13:45].reshape(-1, E)[:4096]
        outs.append(np.concatenate([b_tiles, l3_tiles, l4_tiles], axis=0))
    return np.stack(outs)
